# revision 16
# baseline (speedup 1.0000x reference)
"""Trainium2 Bass kernel for nn_DiffLogicPBF (difflogic network).

Algorithm
---------
The network input is binarized to 2 bits, so every batch row's activation
trajectory takes one of only 4 "patterns".  We evaluate the network on the 4
patterns instead of 8192 rows, then blend per-row.

The per-layer gathers are composed on the host into a stream tree (layer l is
evaluated 2^(5-l) times in permuted orders, 63 streams total), so the device
does gather-free elementwise work only.  Weights arrive pre-permuted, in fp8.

Device pipeline per core (512 neurons x 63 streams = 252 columns of 128):
  exp(w) on ACT  ->  PE matmul with a constant 16->5 matrix that computes the
  multilinear coefficients c0..c3 and the softmax denominator D per neuron
  (one matmul per 1024-neuron chunk, E as the stationary operand so the
  output lands neuron-major)  ->  r = 1/D per column, then one fused
  PSUM->SBUF multiply that both converts and NORMALIZES the coefficients
  (slab = psum * r), so the eval needs no divisions at all  ->  elementwise
  multilinear evaluation over the 4 patterns, 6 tensor_tensor ops per layer.

Sharding: neurons are split across the 8 cores (512 each).  Each core builds
its partial GroupSum table, blends the full batch against it, and the host
sums the 8 partial [B,2] outputs.
"""

from contextlib import ExitStack

import ml_dtypes
import numpy as np

import concourse.bacc as bacc
import concourse.mybir as mybir
import concourse.tile as tile
from concourse.bass_utils import run_bass_kernel_spmd

F32 = mybir.dt.float32
BF16 = mybir.dt.bfloat16
FP8 = mybir.dt.float8e4
ADD = mybir.AluOpType.add
SUB = mybir.AluOpType.subtract
MUL = mybir.AluOpType.mult
GT = mybir.AluOpType.is_gt
X = mybir.AxisListType.X
EXP = mybir.ActivationFunctionType.Exp

N_CORES = 8
B, K, L = 8192, 4096, 6
NS = [32, 16, 8, 4, 2, 1]            # streams per layer
FO = np.cumsum([0] + NS).tolist()    # stream offsets by layer
COLB = [f * 4 for f in FO]           # column base per layer
NCH = 32                             # 8-column chunks (incl. half-chunk of pad)
BROW = B // 128

_compiled = None


def _build_program(u_on_gp=True):
    nc = bacc.Bacc("TRN2", target_bir_lowering=False, debug=False,
                   num_devices=N_CORES)
    wallin = nc.dram_tensor("wallin", [128, 4096], FP8, kind="ExternalInput").ap()
    abin = nc.dram_tensor("abin", [128, 1024], BF16, kind="ExternalInput").ap()
    xkin = nc.dram_tensor("xkin", [128, 170], BF16, kind="ExternalInput").ap()
    out = nc.dram_tensor("out", [B, 2], F32, kind="ExternalOutput").ap()

    with tile.TileContext(nc) as tc:
        with ExitStack() as ctx:
            p = ctx.enter_context(tc.tile_pool(name="p", bufs=1))
            psp = ctx.enter_context(tc.tile_pool(name="ps", bufs=1, space="PSUM"))

            # ---- input DMAs (wall first: it gates the whole pipeline) ----
            wall = p.tile([128, 4096], FP8)
            nc.sync.dma_start(wall[:, 0:1024], wallin[:, 0:1024])
            nc.scalar.dma_start(wall[:, 1024:2048], wallin[:, 1024:2048])
            nc.sync.dma_start(wall[:, 2048:4096], wallin[:, 2048:4096])
            ab = p.tile([128, 1024], BF16)
            nc.gpsimd.dma_start(ab[:], abin[:])
            xk = p.tile([128, 170], BF16)
            nc.scalar.dma_start(xk[:], xkin[:])

            av = ab[:, 0:512].rearrange("p (q c) -> p q c", c=128)
            bv = ab[:, 512:1024].rearrange("p (q c) -> p q c", c=128)
            xv = xk[:, 0:128].rearrange("p (a c) -> p a c", c=2)
            kmv = xk[:, 130:170]

            # ones for the table-broadcast matmul, built on device
            onesb = p.tile([128, 128], BF16)
            nc.gpsimd.memset(onesb[:], 1.0)

            # blend prep on DVE while weights are in flight: one-hot
            # pattern masks m00/m10/m01/m11 per batch row
            s0 = p.tile([128, BROW], F32)
            nc.vector.tensor_scalar(s0[:], xv[:, :, 0], 0.0, None, op0=GT)
            s1 = p.tile([128, BROW], F32)
            nc.vector.tensor_scalar(s1[:], xv[:, :, 1], 0.0, None, op0=GT)
            m11 = p.tile([128, BROW], F32)
            nc.vector.tensor_tensor(m11[:], s0[:], s1[:], op=MUL)
            m10 = p.tile([128, BROW], F32)
            nc.vector.tensor_tensor(m10[:], s0[:], m11[:], op=SUB)
            m01 = p.tile([128, BROW], F32)
            nc.vector.tensor_tensor(m01[:], s1[:], m11[:], op=SUB)
            n1 = p.tile([128, BROW], F32)
            nc.vector.tensor_scalar(n1[:], s0[:], -1.0, 1.0, op0=MUL, op1=ADD)
            m00 = p.tile([128, BROW], F32)
            nc.vector.tensor_tensor(m00[:], n1[:], m01[:], op=SUB)

            # ---- exp on ACT, 4 chunks ----
            E = p.tile([128, 4096], BF16)
            for k in range(4):
                nc.scalar.activation(E[:, k * 1024:(k + 1) * 1024],
                                     wall[:, k * 1024:(k + 1) * 1024], EXP)

            # ---- coefficient matmuls: one per 8-column chunk ----
            psb = [psp.tile([128, 320], F32, tag=f"pb{b}", name=f"pb{b}")
                   for b in range(4)]
            for c in range(NCH):
                b, s = c // 8, c % 8
                nc.tensor.matmul(psb[b][:, s * 40:(s + 1) * 40],
                                 E[:, c * 128:(c + 1) * 128], kmv,
                                 start=True, stop=True)

            # ---- per-bank: r = 1/D, then normalize+convert coefficients ----
            # slabK[p, k, col]: dense NORMALIZED coefficient planes (c_k / D)
            slabK = p.tile([128, 4, 256], BF16)
            rall = p.tile([128, 256], F32)

            def coeff_bank(b):
                # psum chunk layout is g-major: f = g*5 + k, so the D plane
                # is a single stride-5 run and r fits a rank-2 custom-DVE AP
                pv = psb[b][:].rearrange("p (s g k) -> p k s g", k=5, g=8)
                dflat = psb[b][:].rearrange("p (sg k) -> p sg k", k=5)
                rv = rall[:, b * 64:(b + 1) * 64]
                nc.vector.reciprocal_approx_fast(rv, dflat[:, :, 4])
                outv = slabK[:, :, b * 64:(b + 1) * 64].rearrange(
                    "p k (s g) -> p k s g", g=8)
                rb = rall[:, b * 64:(b + 1) * 64].rearrange(
                    "p (s g) -> p s g", g=8).unsqueeze(1).broadcast_to(
                    [128, 4, 8, 8])
                nc.vector.tensor_tensor(outv, pv[:, 0:4], rb, op=MUL)

            H = {}
            for l in range(L):
                n = COLB[l + 1] - COLB[l]
                H[l] = p.tile([128, 4, n], BF16, tag=f"H{l}", name=f"H{l}")

            def bc(apl, n):
                return apl.unsqueeze(1).broadcast_to([128, 4, n])

            u_eng = nc.gpsimd if u_on_gp else nc.vector

            def eval_piece(l, lo, hi, tag):
                # V = [t | u] double-wide: t = B*c3 + c1, u = B*c2 + c0,
                # then H = t*A + u.  slabK plane order is [c3, c2, c1, c0],
                # so each coefficient pair is one strided AP.
                n = hi - lo
                if l == 0:
                    A = av[:, :, lo:hi]
                    BB = bv[:, :, lo:hi].unsqueeze(2).broadcast_to([128, 4, 2, n])
                else:
                    Hp = H[l - 1]
                    A = Hp[:, :, 0:n]
                    BB = Hp[:, :, n:2 * n].unsqueeze(2).broadcast_to(
                        [128, 4, 2, n])
                c32 = slabK[:, 0:2, lo:hi].unsqueeze(1).broadcast_to(
                    [128, 4, 2, n])
                c10 = slabK[:, 2:4, lo:hi].unsqueeze(1).broadcast_to(
                    [128, 4, 2, n])
                llo = lo - COLB[l]
                Hv = H[l][:, :, llo:llo + n]
                V = p.tile([128, 4, 2, n], BF16, tag=f"V{tag}", name=f"V{tag}")
                nc.vector.tensor_tensor(V[:], BB, c32, op=MUL)
                nc.vector.tensor_tensor(V[:], V[:], c10, op=ADD)
                m = p.tile([128, 4, n], BF16, tag=f"m{tag}", name=f"m{tag}")
                nc.vector.tensor_tensor(m[:], V[:, :, 0], A, op=MUL)
                nc.vector.tensor_tensor(Hv, m[:], V[:, :, 1], op=ADD)

            # ---- interleaved schedule ----
            # wait floors keep the scheduler from hoisting later banks'
            # PSUM-gated ops ahead of ready eval ops (DVE head-of-line)
            coeff_bank(0)
            eval_piece(0, 0, 64, "l0a")
            with tc.tile_wait_until(0.008):
                coeff_bank(1)
            eval_piece(0, 64, 128, "l0b")
            with tc.tile_wait_until(0.0095):
                coeff_bank(2)
            eval_piece(1, 128, 192, "l1")
            with tc.tile_wait_until(0.011):
                coeff_bank(3)
            eval_piece(2, 192, 224, "l2")
            eval_piece(3, 224, 240, "l3")
            eval_piece(4, 240, 248, "l4")
            eval_piece(5, 248, 252, "l5")

            # ---- partial GroupSum table (H5 is already normalized) ----
            Hred = p.tile([128, 4], F32)
            nc.vector.tensor_reduce(Hred[:], H[5][:], axis=X, op=ADD)

            gpt = p.tile([128, 4], BF16)
            nc.vector.tensor_copy(gpt[:], Hred[:])

            psg = psp.tile([128, 4], F32)
            nc.tensor.matmul(psg[:], onesb[:], gpt[:], start=True, stop=True)

            # ---- per-row blend: z = sum_q T[q] * mask_q (table from PSUM) ----
            za = p.tile([128, BROW], F32)
            nc.vector.tensor_scalar(za[:], m00[:], psg[:, 0:1], None, op0=MUL)
            w3 = p.tile([128, BROW], F32)
            nc.vector.tensor_scalar(w3[:], m11[:], psg[:, 3:4], None, op0=MUL)
            zb = p.tile([128, BROW], F32)
            nc.vector.scalar_tensor_tensor(zb[:], m10[:], psg[:, 1:2], za[:],
                                           op0=MUL, op1=ADD)
            z2 = p.tile([128, BROW], F32)
            nc.vector.scalar_tensor_tensor(z2[:], m01[:], psg[:, 2:3], w3[:],
                                           op0=MUL, op1=ADD)
            zs = p.tile([128, BROW], F32)
            nc.vector.tensor_tensor(zs[:], zb[:], z2[:], op=ADD)

            osb = p.tile([128, BROW, 2], F32)
            nc.vector.tensor_tensor(osb[:, :, 0], zs[:],
                                    xk[:, 128:129].broadcast_to([128, BROW]),
                                    op=MUL)
            nc.vector.tensor_tensor(osb[:, :, 1], zs[:],
                                    xk[:, 129:130].broadcast_to([128, BROW]),
                                    op=MUL)
            nc.sync.dma_start(out.rearrange("(p a) c -> p a c", p=128), osb[:])

    nc.compile()
    return nc


def _host_blobs(x, w0, ws, idx0, idxs):
    """Compose the stream tree and build per-core input blobs."""
    x = np.asarray(x, np.float32)
    Wl = [np.asarray(w0, np.float32)] + [np.asarray(ws[i], np.float32)
                                         for i in range(L - 1)]
    Il = [np.asarray(idx0, np.int64)] + [np.asarray(idxs[i], np.int64)
                                         for i in range(L - 1)]

    S = [None] * L
    S[L - 1] = [np.arange(K)]
    for l in range(L - 1, 0, -1):
        S[l - 1] = [Il[l][0][P] for P in S[l]] + [Il[l][1][P] for P in S[l]]

    # weights in column order: wtmp[core, col, p, i], col = 4*stream + j
    wtmp = np.zeros((N_CORES, 256, 128, 16), np.float32)
    for l in range(L):
        for s in range(NS[l]):
            sg = FO[l] + s
            pw = Wl[l][S[l][s]].reshape(N_CORES, 4, 128, 16)
            wtmp[:, sg * 4:(sg + 1) * 4] = pw
    # wall[core, g*16+i, c*128+p], col = c*8+g
    wt = wtmp.reshape(N_CORES, 32, 8, 128, 16)
    wall = np.ascontiguousarray(
        wt.transpose(0, 2, 4, 1, 3).reshape(N_CORES, 128, 4096))

    # layer-0 pattern inputs, pattern-major: a0[core, p, q*128 + col]
    q = np.arange(4)
    msel0 = np.zeros((N_CORES, 128, 128), np.int64)  # [core, col, p]
    msel1 = np.zeros((N_CORES, 128, 128), np.int64)
    for s in range(NS[0]):
        idx = S[0][s].reshape(N_CORES, 4, 128)
        msel0[:, s * 4:(s + 1) * 4] = Il[0][0][idx]
        msel1[:, s * 4:(s + 1) * 4] = Il[0][1][idx]
    a0 = (q[None, :, None, None] >> msel0[:, None, :, :]) & 1   # [core,q,col,p]
    b0 = (q[None, :, None, None] >> msel1[:, None, :, :]) & 1
    a0 = a0.transpose(0, 3, 1, 2).reshape(N_CORES, 128, 512)    # [core,p,(q,col)]
    b0 = b0.transpose(0, 3, 1, 2).reshape(N_CORES, 128, 512)

    # constant 16->5 coefficient matrix, block-diagonal over 8 groups
    i16 = np.arange(16)
    t11, t10 = i16 & 1, (i16 >> 1) & 1
    t01, t00 = (i16 >> 2) & 1, (i16 >> 3) & 1
    KC = np.stack([t11 - t10 - t01 + t00, t01 - t00, t10 - t00,
                   t00, np.ones(16, np.int64)], 1)  # [16,5]: c3,c2,c1,c0,D
    kb = np.zeros((8, 16, 8, 5), np.float32)
    for gidx in range(8):
        kb[gidx, :, gidx, :] = KC
    kblob = kb.reshape(128, 40)

    xpart = np.ascontiguousarray(x.reshape(128, 128))

    in_maps = []
    for ci in range(N_CORES):
        cls = np.array([1.0, 0.0] if ci < N_CORES // 2 else [0.0, 1.0],
                       np.float32)
        abm = np.concatenate([a0[ci], b0[ci]], axis=1)
        xkm = np.concatenate([xpart, np.tile(cls, (128, 1)), kblob], axis=1)
        in_maps.append({
            "wallin": wall[ci].astype(ml_dtypes.float8_e4m3fn),
            "abin": np.ascontiguousarray(abm).astype(ml_dtypes.bfloat16),
            "xkin": np.ascontiguousarray(xkm).astype(ml_dtypes.bfloat16),
        })
    return in_maps


def run(inputs, trace=False, trace_kwargs=None):
    global _compiled
    if _compiled is None:
        _compiled = _build_program()
    nc = _compiled
    in_maps = _host_blobs(inputs["x"], inputs["w0"], inputs["ws"],
                          inputs["idx0"], inputs["idxs"])
    res = run_bass_kernel_spmd(nc, in_maps, core_ids=list(range(N_CORES)),
                               trace=trace, **(trace_kwargs or {}))
    total = np.zeros((B, 2), np.float32)
    for ci in range(N_CORES):
        total += res.results[ci]["out"]
    return total, res


def kernel(x, w0, ws, idx0, idxs):
    out, _ = run({"x": x, "w0": w0, "ws": ws, "idx0": idx0, "idxs": idxs})
    return out


# revision 17
# speedup vs baseline: 1.0630x; 1.0630x over previous
"""Trainium2 Bass kernel for nn_DiffLogicPBF (difflogic network).

Algorithm
---------
The network input is binarized to 2 bits, so every batch row's activation
trajectory takes one of only 4 "patterns".  We evaluate the network on the 4
patterns instead of 8192 rows, then blend per-row.

The per-layer gathers are composed on the host into a stream tree (layer l is
evaluated 2^(5-l) times in permuted orders, 63 streams total), so the device
does gather-free elementwise work only.  Weights arrive pre-permuted, in fp8.

Device pipeline per core (512 neurons x 63 streams = 252 columns of 128):
  exp(w) on ACT  ->  PE matmul with a constant 16->5 matrix that computes the
  multilinear coefficients c0..c3 and the softmax denominator D per neuron
  (one matmul per 1024-neuron chunk, E as the stationary operand so the
  output lands neuron-major)  ->  r = 1/D per column, then one fused
  PSUM->SBUF multiply that both converts and NORMALIZES the coefficients
  (slab = psum * r), so the eval needs no divisions at all  ->  elementwise
  multilinear evaluation over the 4 patterns, 6 tensor_tensor ops per layer.

Sharding: neurons are split across the 8 cores (512 each).  Each core builds
its partial GroupSum table, blends the full batch against it, and the host
sums the 8 partial [B,2] outputs.
"""

from contextlib import ExitStack

import ml_dtypes
import numpy as np

import concourse.bacc as bacc
import concourse.mybir as mybir
import concourse.tile as tile
from concourse.bass_utils import run_bass_kernel_spmd

F32 = mybir.dt.float32
BF16 = mybir.dt.bfloat16
FP8 = mybir.dt.float8e4
ADD = mybir.AluOpType.add
SUB = mybir.AluOpType.subtract
MUL = mybir.AluOpType.mult
GT = mybir.AluOpType.is_gt
X = mybir.AxisListType.X
EXP = mybir.ActivationFunctionType.Exp

N_CORES = 8
B, K, L = 8192, 4096, 6
NS = [32, 16, 8, 4, 2, 1]            # streams per layer
FO = np.cumsum([0] + NS).tolist()    # stream offsets by layer
COLB = [f * 4 for f in FO]           # column base per layer
NCH = 32                             # 8-column chunks (incl. half-chunk of pad)
BROW = B // 128

_compiled = None


def _build_program(u_on_gp=True):
    nc = bacc.Bacc("TRN2", target_bir_lowering=False, debug=False,
                   num_devices=N_CORES)
    wallin = nc.dram_tensor("wallin", [128, 4096], FP8, kind="ExternalInput").ap()
    abin = nc.dram_tensor("abin", [128, 1024], BF16, kind="ExternalInput").ap()
    xkin = nc.dram_tensor("xkin", [128, 170], BF16, kind="ExternalInput").ap()
    out = nc.dram_tensor("out", [B, 2], F32, kind="ExternalOutput").ap()

    with tile.TileContext(nc) as tc:
        with ExitStack() as ctx:
            p = ctx.enter_context(tc.tile_pool(name="p", bufs=1))
            psp = ctx.enter_context(tc.tile_pool(name="ps", bufs=1, space="PSUM"))

            # ---- input DMAs (wall first: it gates the whole pipeline) ----
            wall = p.tile([128, 4096], FP8)
            nc.sync.dma_start(wall[:, 0:1024], wallin[:, 0:1024])
            nc.sync.dma_start(wall[:, 1024:2048], wallin[:, 1024:2048])
            nc.sync.dma_start(wall[:, 2048:4096], wallin[:, 2048:4096])
            ab = p.tile([128, 1024], BF16)
            nc.gpsimd.dma_start(ab[:], abin[:])
            xk = p.tile([128, 170], BF16)
            nc.scalar.dma_start(xk[:], xkin[:])

            av = ab[:, 0:512].rearrange("p (q c) -> p q c", c=128)
            bv = ab[:, 512:1024].rearrange("p (q c) -> p q c", c=128)
            xv = xk[:, 0:128].rearrange("p (a c) -> p a c", c=2)
            kmv = xk[:, 130:170]

            # ones for the table-broadcast matmul, built on device
            onesb = p.tile([128, 128], BF16)
            nc.gpsimd.memset(onesb[:], 1.0)

            # blend prep on DVE while weights are in flight: one-hot
            # pattern masks m00/m10/m01/m11 per batch row
            s0 = p.tile([128, BROW], F32)
            nc.vector.tensor_scalar(s0[:], xv[:, :, 0], 0.0, None, op0=GT)
            s1 = p.tile([128, BROW], F32)
            nc.vector.tensor_scalar(s1[:], xv[:, :, 1], 0.0, None, op0=GT)
            m11 = p.tile([128, BROW], F32)
            nc.vector.tensor_tensor(m11[:], s0[:], s1[:], op=MUL)
            m10 = p.tile([128, BROW], F32)
            nc.vector.tensor_tensor(m10[:], s0[:], m11[:], op=SUB)
            m01 = p.tile([128, BROW], F32)
            nc.vector.tensor_tensor(m01[:], s1[:], m11[:], op=SUB)
            n1 = p.tile([128, BROW], F32)
            nc.vector.tensor_scalar(n1[:], s0[:], -1.0, 1.0, op0=MUL, op1=ADD)
            m00 = p.tile([128, BROW], F32)
            nc.vector.tensor_tensor(m00[:], n1[:], m01[:], op=SUB)

            # ---- exp on ACT, 4 chunks ----
            E = p.tile([128, 4096], BF16)
            for k in range(4):
                nc.scalar.activation(E[:, k * 1024:(k + 1) * 1024],
                                     wall[:, k * 1024:(k + 1) * 1024], EXP)

            # ---- coefficient matmuls: one per 8-column chunk ----
            psb = [psp.tile([128, 320], F32, tag=f"pb{b}", name=f"pb{b}")
                   for b in range(4)]
            for c in range(NCH):
                b, s = c // 8, c % 8
                nc.tensor.matmul(psb[b][:, s * 40:(s + 1) * 40],
                                 E[:, c * 128:(c + 1) * 128], kmv,
                                 start=True, stop=True)

            # ---- per-bank: r = 1/D, then normalize+convert coefficients ----
            # slabK[p, k, col]: dense NORMALIZED coefficient planes (c_k / D)
            slabK = p.tile([128, 4, 256], BF16)
            rall = p.tile([128, 256], F32)

            def coeff_bank(b):
                # psum chunk layout is g-major: f = g*5 + k, so the D plane
                # is a single stride-5 run and r fits a rank-2 custom-DVE AP
                pv = psb[b][:].rearrange("p (s g k) -> p k s g", k=5, g=8)
                dflat = psb[b][:].rearrange("p (sg k) -> p sg k", k=5)
                rv = rall[:, b * 64:(b + 1) * 64]
                nc.vector.reciprocal_approx_fast(rv, dflat[:, :, 4])
                outv = slabK[:, :, b * 64:(b + 1) * 64].rearrange(
                    "p k (s g) -> p k s g", g=8)
                rb = rall[:, b * 64:(b + 1) * 64].rearrange(
                    "p (s g) -> p s g", g=8).unsqueeze(1).broadcast_to(
                    [128, 4, 8, 8])
                nc.vector.tensor_tensor(outv, pv[:, 0:4], rb, op=MUL)

            H = {}
            for l in range(L):
                n = COLB[l + 1] - COLB[l]
                H[l] = p.tile([128, 4, n], BF16, tag=f"H{l}", name=f"H{l}")

            def bc(apl, n):
                return apl.unsqueeze(1).broadcast_to([128, 4, n])

            u_eng = nc.gpsimd if u_on_gp else nc.vector

            def eval_piece(l, lo, hi, tag):
                # V = [t | u] double-wide: t = B*c3 + c1, u = B*c2 + c0,
                # then H = t*A + u.  slabK plane order is [c3, c2, c1, c0],
                # so each coefficient pair is one strided AP.
                n = hi - lo
                if l == 0:
                    A = av[:, :, lo:hi]
                    BB = bv[:, :, lo:hi].unsqueeze(2).broadcast_to([128, 4, 2, n])
                else:
                    Hp = H[l - 1]
                    A = Hp[:, :, 0:n]
                    BB = Hp[:, :, n:2 * n].unsqueeze(2).broadcast_to(
                        [128, 4, 2, n])
                c32 = slabK[:, 0:2, lo:hi].unsqueeze(1).broadcast_to(
                    [128, 4, 2, n])
                c10 = slabK[:, 2:4, lo:hi].unsqueeze(1).broadcast_to(
                    [128, 4, 2, n])
                llo = lo - COLB[l]
                Hv = H[l][:, :, llo:llo + n]
                V = p.tile([128, 4, 2, n], BF16, tag=f"V{tag}", name=f"V{tag}")
                nc.vector.tensor_tensor(V[:], BB, c32, op=MUL)
                nc.vector.tensor_tensor(V[:], V[:], c10, op=ADD)
                m = p.tile([128, 4, n], BF16, tag=f"m{tag}", name=f"m{tag}")
                nc.vector.tensor_tensor(m[:], V[:, :, 0], A, op=MUL)
                nc.vector.tensor_tensor(Hv, m[:], V[:, :, 1], op=ADD)

            # ---- interleaved schedule ----
            # wait floors keep the scheduler from hoisting later banks'
            # PSUM-gated ops ahead of ready eval ops (DVE head-of-line)
            coeff_bank(0)
            eval_piece(0, 0, 64, "l0a")
            with tc.tile_wait_until(0.008):
                coeff_bank(1)
            eval_piece(0, 64, 128, "l0b")
            with tc.tile_wait_until(0.0095):
                coeff_bank(2)
            eval_piece(1, 128, 192, "l1")
            with tc.tile_wait_until(0.011):
                coeff_bank(3)
            eval_piece(2, 192, 224, "l2")
            eval_piece(3, 224, 240, "l3")
            eval_piece(4, 240, 248, "l4")
            eval_piece(5, 248, 252, "l5")

            # ---- partial GroupSum table (H5 is already normalized) ----
            Hred = p.tile([128, 4], F32)
            nc.vector.tensor_reduce(Hred[:], H[5][:], axis=X, op=ADD)

            gpt = p.tile([128, 4], BF16)
            nc.vector.tensor_copy(gpt[:], Hred[:])

            psg = psp.tile([128, 4], F32)
            nc.tensor.matmul(psg[:], onesb[:], gpt[:], start=True, stop=True)

            # ---- per-row blend: z = sum_q T[q] * mask_q (table from PSUM) ----
            za = p.tile([128, BROW], F32)
            nc.vector.tensor_scalar(za[:], m00[:], psg[:, 0:1], None, op0=MUL)
            w3 = p.tile([128, BROW], F32)
            nc.vector.tensor_scalar(w3[:], m11[:], psg[:, 3:4], None, op0=MUL)
            zb = p.tile([128, BROW], F32)
            nc.vector.scalar_tensor_tensor(zb[:], m10[:], psg[:, 1:2], za[:],
                                           op0=MUL, op1=ADD)
            z2 = p.tile([128, BROW], F32)
            nc.vector.scalar_tensor_tensor(z2[:], m01[:], psg[:, 2:3], w3[:],
                                           op0=MUL, op1=ADD)
            zs = p.tile([128, BROW], F32)
            nc.vector.tensor_tensor(zs[:], zb[:], z2[:], op=ADD)

            osb = p.tile([128, BROW, 2], F32)
            nc.vector.tensor_tensor(osb[:, :, 0], zs[:],
                                    xk[:, 128:129].broadcast_to([128, BROW]),
                                    op=MUL)
            nc.vector.tensor_tensor(osb[:, :, 1], zs[:],
                                    xk[:, 129:130].broadcast_to([128, BROW]),
                                    op=MUL)
            nc.sync.dma_start(out.rearrange("(p a) c -> p a c", p=128), osb[:])

    nc.compile()
    return nc


def _host_blobs(x, w0, ws, idx0, idxs):
    """Compose the stream tree and build per-core input blobs."""
    x = np.asarray(x, np.float32)
    Wl = [np.asarray(w0, np.float32)] + [np.asarray(ws[i], np.float32)
                                         for i in range(L - 1)]
    Il = [np.asarray(idx0, np.int64)] + [np.asarray(idxs[i], np.int64)
                                         for i in range(L - 1)]

    S = [None] * L
    S[L - 1] = [np.arange(K)]
    for l in range(L - 1, 0, -1):
        S[l - 1] = [Il[l][0][P] for P in S[l]] + [Il[l][1][P] for P in S[l]]

    # weights in column order: wtmp[core, col, p, i], col = 4*stream + j
    wtmp = np.zeros((N_CORES, 256, 128, 16), np.float32)
    for l in range(L):
        for s in range(NS[l]):
            sg = FO[l] + s
            pw = Wl[l][S[l][s]].reshape(N_CORES, 4, 128, 16)
            wtmp[:, sg * 4:(sg + 1) * 4] = pw
    # wall[core, g*16+i, c*128+p], col = c*8+g
    wt = wtmp.reshape(N_CORES, 32, 8, 128, 16)
    wall = np.ascontiguousarray(
        wt.transpose(0, 2, 4, 1, 3).reshape(N_CORES, 128, 4096))

    # layer-0 pattern inputs, pattern-major: a0[core, p, q*128 + col]
    q = np.arange(4)
    msel0 = np.zeros((N_CORES, 128, 128), np.int64)  # [core, col, p]
    msel1 = np.zeros((N_CORES, 128, 128), np.int64)
    for s in range(NS[0]):
        idx = S[0][s].reshape(N_CORES, 4, 128)
        msel0[:, s * 4:(s + 1) * 4] = Il[0][0][idx]
        msel1[:, s * 4:(s + 1) * 4] = Il[0][1][idx]
    a0 = (q[None, :, None, None] >> msel0[:, None, :, :]) & 1   # [core,q,col,p]
    b0 = (q[None, :, None, None] >> msel1[:, None, :, :]) & 1
    a0 = a0.transpose(0, 3, 1, 2).reshape(N_CORES, 128, 512)    # [core,p,(q,col)]
    b0 = b0.transpose(0, 3, 1, 2).reshape(N_CORES, 128, 512)

    # constant 16->5 coefficient matrix, block-diagonal over 8 groups
    i16 = np.arange(16)
    t11, t10 = i16 & 1, (i16 >> 1) & 1
    t01, t00 = (i16 >> 2) & 1, (i16 >> 3) & 1
    KC = np.stack([t11 - t10 - t01 + t00, t01 - t00, t10 - t00,
                   t00, np.ones(16, np.int64)], 1)  # [16,5]: c3,c2,c1,c0,D
    kb = np.zeros((8, 16, 8, 5), np.float32)
    for gidx in range(8):
        kb[gidx, :, gidx, :] = KC
    kblob = kb.reshape(128, 40)

    xpart = np.ascontiguousarray(x.reshape(128, 128))

    in_maps = []
    for ci in range(N_CORES):
        cls = np.array([1.0, 0.0] if ci < N_CORES // 2 else [0.0, 1.0],
                       np.float32)
        abm = np.concatenate([a0[ci], b0[ci]], axis=1)
        xkm = np.concatenate([xpart, np.tile(cls, (128, 1)), kblob], axis=1)
        in_maps.append({
            "wallin": wall[ci].astype(ml_dtypes.float8_e4m3fn),
            "abin": np.ascontiguousarray(abm).astype(ml_dtypes.bfloat16),
            "xkin": np.ascontiguousarray(xkm).astype(ml_dtypes.bfloat16),
        })
    return in_maps


def run(inputs, trace=False, trace_kwargs=None):
    global _compiled
    if _compiled is None:
        _compiled = _build_program()
    nc = _compiled
    in_maps = _host_blobs(inputs["x"], inputs["w0"], inputs["ws"],
                          inputs["idx0"], inputs["idxs"])
    res = run_bass_kernel_spmd(nc, in_maps, core_ids=list(range(N_CORES)),
                               trace=trace, **(trace_kwargs or {}))
    total = np.zeros((B, 2), np.float32)
    for ci in range(N_CORES):
        total += res.results[ci]["out"]
    return total, res


def kernel(x, w0, ws, idx0, idxs):
    out, _ = run({"x": x, "w0": w0, "ws": ws, "idx0": idx0, "idxs": idxs})
    return out
